# revision 8
# baseline (speedup 1.0000x reference)
"""Trainium2 Bass kernel: 3D max pooling (kernel=2, stride=2, pad=0).

Input  x: (2, 32, 96, 96, 96) f32  ->  Output: (2, 32, 48, 48, 48) f32.

Sharding: data-parallel over the 64 (N,C) volumes -> 8 volumes per core.

Per-core layout (volumes processed in pairs, packed into 96 SBUF partitions):
  - partition dim = (vol in pair, even/odd D-plane index d2) -> 96 rows
  - D-pooling: load even-d planes with a plain DMA, then odd-d planes with a
    SWDGE accum DMA (CCE max) into the same tile -> no DVE work for the
    largest reduction stage (optionally done on DVE instead, see use_accum).
  - W-pooling then H-pooling as strided tensor_tensor max ops on the DVE.
  - H dimension is chunked so tiles stay small and the pipeline overlaps.
"""

import sys

sys.path.insert(0, "/opt/trn_rl_repo")

import numpy as np

from concourse import bacc, mybir, tile
from concourse.bass_utils import run_bass_kernel_spmd

N_CORES = 8
VPC = 8  # volumes per core (64 total / 8 cores)
D = H = W = 96
DO = HO = WO = 48
DT = mybir.dt.float32


def _build(use_accum=False, hc=48, bufs=3, repeat=1, dchunk=16):
    """Build the SPMD Bass program for one core: x[8,96,96,96] -> out[8,48,48,48].

    Partition layout: (vol 0..7) x (d2 chunk of `dchunk`) = 8*dchunk SBUF
    partitions per tile (dchunk=16 -> all 128). Free dim = (h chunk, w).

    repeat>1 re-runs the whole kernel body R times (same I/O) — used only for
    slope-based wall-clock benchmarking, never for the graded call.
    """
    nc = bacc.Bacc("TRN2", target_bir_lowering=False, debug=False, num_devices=N_CORES)
    x = nc.dram_tensor("x", [VPC, D, H, W], DT, kind="ExternalInput").ap()
    o = nc.dram_tensor("out", [VPC, DO, HO, WO], DT, kind="ExternalOutput").ap()

    # [8, 48, 2, 96, 96]: (vol, d2, even/odd, h, w)
    xe = x.rearrange("v (d two) h w -> v d two h w", two=2)

    nchunk = H // hc
    ndchunk = DO // dchunk
    npart = VPC * dchunk
    assert hc % 2 == 0 and H % hc == 0 and DO % dchunk == 0 and npart <= 128

    with tile.TileContext(nc) as tc:
        with tc.tile_pool(name="pool", bufs=bufs) as pool:
            for rep in range(repeat):
                for j in range(ndchunk):  # d2 chunk
                    d0 = j * dchunk
                    for ci in range(nchunk):  # h chunk
                        h0 = ci * hc

                        # ---- load + D-pool ----
                        # One DMA per volume keeps every AP 2D (single
                        # partition dim) — multi-dim partition APs mislower
                        # on HW. Rows 16v..16v+16 of the tile = volume v.
                        tm = pool.tile([128, hc * W], DT, tag="tm")
                        tmv = tm[0:npart, :]
                        if use_accum:
                            for v in range(VPC):
                                dst = tm[v * dchunk : (v + 1) * dchunk, :]
                                nc.sync.dma_start(
                                    out=dst,
                                    in_=xe[v, d0 : d0 + dchunk, 0, h0 : h0 + hc, :].opt(),
                                )
                                nc.gpsimd.dma_start(
                                    out=dst,
                                    in_=xe[v, d0 : d0 + dchunk, 1, h0 : h0 + hc, :].opt(),
                                    accum_op=mybir.AluOpType.max,
                                )
                        else:
                            te = pool.tile([128, hc * W], DT, tag="te")
                            tev = te[0:npart, :]
                            for v in range(VPC):
                                nc.sync.dma_start(
                                    out=tm[v * dchunk : (v + 1) * dchunk, :],
                                    in_=xe[v, d0 : d0 + dchunk, 0, h0 : h0 + hc, :].opt(),
                                )
                                nc.sync.dma_start(
                                    out=te[v * dchunk : (v + 1) * dchunk, :],
                                    in_=xe[v, d0 : d0 + dchunk, 1, h0 : h0 + hc, :].opt(),
                                )
                            nc.vector.tensor_max(tmv, tmv, tev)

                        # ---- W-pool: [P, hc, 96] -> [P, hc, 48] ----
                        tw = pool.tile([128, hc * WO], DT, tag="tw")
                        twv = tw[0:npart, :].rearrange("p (h w) -> p h w", h=hc)
                        mv = tmv.rearrange("p (h w two) -> p h w two", h=hc, two=2)
                        nc.vector.tensor_max(twv, mv[:, :, :, 0], mv[:, :, :, 1])

                        # ---- H-pool: [P, hc/2, 2, 48] -> [P, hc/2, 48] ----
                        th = pool.tile([128, (hc // 2) * WO], DT, tag="th")
                        thv = th[0:npart, :].rearrange(
                            "p (h w) -> p h w", h=hc // 2
                        )
                        wv = tw[0:npart, :].rearrange(
                            "p (h two w) -> p h two w", two=2, w=WO
                        )
                        nc.vector.tensor_max(thv, wv[:, :, 0, :], wv[:, :, 1, :])

                        # ---- store (per volume, 2D APs) ----
                        for v in range(VPC):
                            nc.sync.dma_start(
                                out=o[
                                    v, d0 : d0 + dchunk, h0 // 2 : (h0 + hc) // 2, :
                                ].opt(),
                                in_=th[v * dchunk : (v + 1) * dchunk, :],
                            )

    nc.finalize()
    return nc


_NC_CACHE = {}


def _get_nc(**kw):
    key = tuple(sorted(kw.items()))
    if key not in _NC_CACHE:
        _NC_CACHE[key] = _build(**kw)
    return _NC_CACHE[key]


def _run(x, trace=False, **build_kw):
    assert x.shape == (2, 32, 96, 96, 96) and x.dtype == np.float32
    nc = _get_nc(**build_kw)
    xs = np.ascontiguousarray(x.reshape(64, D, H, W))
    in_maps = [{"x": xs[i * VPC : (i + 1) * VPC]} for i in range(N_CORES)]
    res = run_bass_kernel_spmd(nc, in_maps, core_ids=list(range(N_CORES)), trace=trace)
    out = np.concatenate([res.results[i]["out"] for i in range(N_CORES)], axis=0)
    return out.reshape(2, 32, DO, HO, WO), res


def kernel(x):
    out, _ = _run(np.asarray(x))
    return out


def _bench(x, r_lo=1, r_hi=33, calls=8, **build_kw):
    """Slope-based device timing: run the kernel body R times inside one NEFF
    for R in {r_lo, r_hi}; per-kernel time = (T_hi - T_lo) / (r_hi - r_lo).
    Inputs are device-resident and outputs are not donated, so per-call host
    overhead is identical between the two variants and cancels.
    """
    import time

    import jax
    import jax.numpy as jnp
    from jax.sharding import Mesh, PartitionSpec
    from jax.experimental.shard_map import shard_map

    from concourse import bass2jax, mybir as mb

    bass2jax.install_neuronx_cc_hook()

    xs = np.ascontiguousarray(np.asarray(x).reshape(64, D, H, W))
    devices = jax.devices()[:N_CORES]
    mesh = Mesh(np.asarray(devices), ("core",))

    def make_fn(nc):
        part_name = nc.partition_id_tensor.name if nc.partition_id_tensor else None
        in_names, out_names, out_avals, zero_outs = [], [], [], []
        for alloc in nc.m.functions[0].allocations:
            if not isinstance(alloc, mb.MemoryLocationSet):
                continue
            name = alloc.memorylocations[0].name
            if alloc.kind == "ExternalInput":
                if name != part_name:
                    in_names.append(name)
            elif alloc.kind == "ExternalOutput":
                out_names.append(name)
                shape = tuple(alloc.tensor_shape)
                dtype = mb.dt.np(alloc.dtype)
                out_avals.append(jax.core.ShapedArray(shape, dtype))
                zero_outs.append(np.zeros(shape, dtype))
        n_params = len(in_names)
        all_names = in_names + out_names
        if part_name is not None:
            all_names = all_names + [part_name]

        def _body(*args):
            operands = list(args)
            if part_name is not None:
                operands.append(bass2jax.partition_id_tensor())
            outs = bass2jax._bass_exec_p.bind(
                *operands,
                out_avals=tuple(out_avals),
                in_names=tuple(all_names),
                out_names=tuple(out_names),
                lowering_input_output_aliases=(),
                sim_require_finite=True,
                sim_require_nnan=True,
                nc=nc,
            )
            return tuple(outs)

        in_specs = (PartitionSpec("core"),) * (n_params + len(out_names))
        out_specs = (PartitionSpec("core"),) * len(out_names)
        fn = jax.jit(
            shard_map(
                _body, mesh=mesh, in_specs=in_specs, out_specs=out_specs,
                check_rep=False,
            ),
            keep_unused=True,
        )
        return fn, zero_outs

    results = {}
    for r in (r_lo, r_hi):
        nc = _build(repeat=r, **build_kw)
        fn, zero_outs = make_fn(nc)
        dev_in = jax.device_put(xs, jax.sharding.NamedSharding(mesh, PartitionSpec("core")))
        dev_zeros = [
            jax.device_put(
                np.zeros((N_CORES * z.shape[0], *z.shape[1:]), z.dtype),
                jax.sharding.NamedSharding(mesh, PartitionSpec("core")),
            )
            for z in zero_outs
        ]
        out = fn(dev_in, *dev_zeros)  # warmup + compile
        jax.block_until_ready(out)
        times = []
        for _ in range(calls):
            t0 = time.perf_counter()
            out = fn(dev_in, *dev_zeros)
            jax.block_until_ready(out)
            times.append(time.perf_counter() - t0)
        results[r] = (min(times), sorted(times)[len(times) // 2], out)

    t_lo, m_lo, _ = results[r_lo]
    t_hi, m_hi, out = results[r_hi]
    per_kernel_ns = (t_hi - t_lo) / (r_hi - r_lo) * 1e9
    per_kernel_med_ns = (m_hi - m_lo) / (r_hi - r_lo) * 1e9
    full = np.asarray(out[0]).reshape(2, 32, DO, HO, WO)
    return per_kernel_ns, per_kernel_med_ns, (t_lo, t_hi, m_lo, m_hi), full
